# revision 11
# baseline (speedup 1.0000x reference)
"""Trainium2 Bass kernel: row-wise log-sparsemax via bisection.

Contract: kernel(X) takes the FULL input X [4096, 32000] f32 and returns the
FULL output [4096, 32000] f32.  Rows are sharded across 8 NeuronCores (512
rows each); each core runs an identical (SPMD) Bass program on its slice.

Algorithm (per row, replicating the reference's f32 bisection bitwise up to
reduction order):
  - Only elements within 1.0 of the row max can ever contribute to a
    bisection evaluation f(tau) = sum(relu(x - tau)) - 1, since every
    evaluated tau lies in (rowmax-1, rowmax).  Extract top-8 values per
    1000-column subchunk (vector-engine max8) -> 256 candidates/row, a
    superset of everything that matters.
  - Run the bisection recurrence on candidates only.  In f32 the recurrence
    converges bitwise by iteration ~24 (once dm < ulp(tau)/2, tau_lo + dm
    rounds to tau_lo), so 28 iterations reproduce the reference's 50.
  - Second streaming pass: out = Ln(relu(x - tau) * (1/s)), with
    relu fused on the vector engine (tensor_scalar sub+max, in-place) and
    Ln on the scalar engine (activation with per-partition scale).
    Ln(0) = -inf matches the reference's masked log exactly.

Taus are tracked negated (ntau = -tau) so activation bias (which adds) can
apply them; all negated ops are bitwise-equivalent under IEEE sign symmetry.
"""

import numpy as np

import concourse.bacc as bacc
import concourse.tile as tile
from concourse import mybir

F32 = mybir.dt.float32
AX = mybir.AxisListType
OP = mybir.AluOpType
ACTF = mybir.ActivationFunctionType

N_CORES = 8
ROWS = 4096
COLS = 32000
ROWS_PER_CORE = ROWS // N_CORES  # 512
P = 128
N_ITER = 26


def build_program(rows=ROWS_PER_CORE, cols=COLS, ch=4000, sub=2000,
                  n_iter=N_ITER, bufs_x=10, num_devices=N_CORES, repeat=1,
                  copy_only=False, relu_act=False):
    assert rows % P == 0 and cols % ch == 0 and ch % sub == 0
    n_blocks = rows // P
    n_chunks = cols // ch
    n_sub = ch // sub
    n_cand = (cols // sub) * 8
    inv_d = float(np.float32(1.0 / cols))

    nc = bacc.Bacc("TRN2", target_bir_lowering=False, debug=False,
                   enable_asserts=False, num_devices=num_devices)
    x_d = nc.dram_tensor("X", (rows, cols), F32, kind="ExternalInput").ap()
    o_d = nc.dram_tensor("out", (rows, cols), F32, kind="ExternalOutput").ap()

    with tile.TileContext(nc) as tc:
        with tc.tile_pool(name="xp", bufs=bufs_x) as xp, \
             tc.tile_pool(name="cp", bufs=3) as cpool, \
             tc.tile_pool(name="zp", bufs=1) as zp, \
             tc.tile_pool(name="sp", bufs=4) as sp:
            zeros = zp.tile([P, n_cand], F32)
            nc.vector.memset(zeros[:], 0.0)
            neginf = zp.tile([P, 1], F32)
            nc.vector.memset(neginf[:], float("-inf"))
            for b in range(n_blocks * repeat):
                b = b % n_blocks
                r0 = b * P
                if copy_only:
                    for c in range(n_chunks):
                        xt = xp.tile([P, ch], F32)
                        nc.sync.dma_start(xt[:], x_d[r0:r0 + P, c * ch:(c + 1) * ch])
                        nc.scalar.dma_start(o_d[r0:r0 + P, c * ch:(c + 1) * ch],
                                            xt[:])
                    continue
                cand = cpool.tile([P, n_cand], F32)
                chunks = []
                for c in range(n_chunks):
                    xt = xp.tile([P, ch], F32)
                    nc.sync.dma_start(xt[:], x_d[r0:r0 + P, c * ch:(c + 1) * ch])
                    for j in range(n_sub):
                        k = c * n_sub + j
                        nc.vector.max(cand[:, k * 8:(k + 1) * 8],
                                      xt[:, j * sub:(j + 1) * sub])
                    chunks.append(xt)

                # Row stats in negated-tau space: ntau_lo = 1 - mx,
                # ntau_hi = inv_d - mx, dm0 = ntau_lo - ntau_hi (= tau_hi - tau_lo).
                mx = sp.tile([P, 1], F32)
                nc.vector.tensor_reduce(mx[:], cand[:], AX.X, OP.max)
                ntau_lo = sp.tile([P, 1], F32)
                nc.vector.tensor_scalar(ntau_lo[:], mx[:], -1.0, 1.0,
                                        OP.mult, OP.add)
                ntau_hi = sp.tile([P, 1], F32)
                nc.vector.tensor_scalar(ntau_hi[:], mx[:], -1.0, inv_d,
                                        OP.mult, OP.add)
                dm0 = sp.tile([P, 1], F32)
                nc.vector.tensor_sub(dm0[:], ntau_lo[:], ntau_hi[:])

                # f_lo = sum(relu(cand - tau_lo)) - 1  (raw accum, then -1)
                trash = sp.tile([P, n_cand], F32)
                flo_raw = sp.tile([P, 1], F32)
                nc.vector.scalar_tensor_tensor(trash[:], cand[:], ntau_lo[:],
                                               zeros[:], OP.add, OP.max,
                                               accum_out=flo_raw[:])
                # move-test threshold: fs >= 1.0 normally; f_lo == 0 (i.e.
                # raw sum == 1) means "always move" -> threshold -inf.
                # (f_lo < 0 is impossible: sum of nonneg f32 terms, one of
                # which is exactly 1.0, is >= 1.0 in any order.)
                thresh = sp.tile([P, 1], F32)
                nc.vector.memset(thresh[:], 1.0)
                mk0 = sp.tile([P, 1], mybir.dt.int32)
                nc.vector.tensor_scalar(mk0[:], flo_raw[:], 1.0, None, OP.is_le)
                nc.vector.copy_predicated(thresh[:], mk0[:], neginf[:])

                ntau_m = sp.tile([P, 1], F32)
                fs = sp.tile([P, 1], F32)
                mk = sp.tile([P, 1], mybir.dt.int32)
                for i in range(1, n_iter + 1):
                    # ntau_m = ntau_lo - dm0 * 2^-i  (== -(tau_lo + dm_i))
                    nc.vector.scalar_tensor_tensor(ntau_m[:], dm0[:],
                                                   -(2.0 ** -i), ntau_lo[:],
                                                   OP.mult, OP.add)
                    # f_m_raw = sum(relu(cand - tau_m))
                    nc.vector.scalar_tensor_tensor(trash[:], cand[:],
                                                   ntau_m[:], zeros[:],
                                                   OP.add, OP.max,
                                                   accum_out=fs[:])
                    # move iff f_m_raw >= thresh  (== (f_m*f_lo >= 0))
                    nc.vector.tensor_scalar(mk[:], fs[:], thresh[:], None,
                                            OP.is_ge)
                    nc.vector.copy_predicated(ntau_lo[:], mk[:], ntau_m[:])

                # s = sum(relu(cand - tau_final)); r = 1/s
                ssum = sp.tile([P, 1], F32)
                nc.vector.scalar_tensor_tensor(trash[:], cand[:], ntau_m[:],
                                               zeros[:], OP.add, OP.max,
                                               accum_out=ssum[:])
                rr = sp.tile([P, 1], F32)
                nc.vector.reciprocal(rr[:], ssum[:])

                # Output pass: p = relu(x - tau) in-place, out = Ln(p * r).
                for c in range(n_chunks):
                    xt = chunks[c]
                    if relu_act:
                        nc.scalar.activation(xt[:], xt[:], ACTF.Relu,
                                             bias=ntau_m[:], scale=1.0)
                    else:
                        nc.vector.tensor_scalar(xt[:], xt[:], ntau_m[:], 0.0,
                                                OP.add, OP.max)
                    nc.scalar.activation(xt[:], xt[:], ACTF.Ln,
                                         bias=0.0, scale=rr[:])
                    nc.scalar.dma_start(o_d[r0:r0 + P, c * ch:(c + 1) * ch],
                                        xt[:])
    nc.finalize()
    return nc


_CACHE = {}


def _program():
    if "nc" not in _CACHE:
        _CACHE["nc"] = build_program()
    return _CACHE["nc"]


def kernel(X):
    from concourse.bass_utils import run_bass_kernel_spmd

    X = np.asarray(X)
    assert X.shape == (ROWS, COLS), X.shape
    if X.dtype != np.float32:
        X = X.astype(np.float32)
    nc = _program()
    rpc = ROWS_PER_CORE
    in_maps = [{"X": np.ascontiguousarray(X[c * rpc:(c + 1) * rpc])}
               for c in range(N_CORES)]
    res = run_bass_kernel_spmd(nc, in_maps, core_ids=list(range(N_CORES)))
    return np.concatenate(
        [np.asarray(res.results[c]["out"]) for c in range(N_CORES)], axis=0)
